# revision 20
# baseline (speedup 1.0000x reference)
"""Trainium2 Bass kernel for nn_BaseEncoder (ragged entity-pair encoder).

Contract: kernel(**inputs) takes the FULL unsharded inputs (numpy) and
returns the FULL output [B, Q, E, E, R] float32.

Sharding: B*Q = 8 independent (batch, query) pairs -> one per NeuronCore.

Diagonal-band formulation (v2): the pair tensor mul[l, e, f] is symmetric in
(e, f), so the device computes it only on 544 unique "band" slots:
  main band: slot (d, k), d in 0..15, k in 0..31  -> pair (k, (k+d) % 32)
  strip:     slot k,      k in 0..31              -> pair (k, (k+16) % 32)
Band products use overlapping-window / outer-broadcast access patterns whose
inner stride is 1, which keeps the DVE in its 2x bf16 perf mode (the previous
grid formulation's inner-stride-0 broadcasts ran at 1x).  ctx / S / norm /
proj all run once on the shared symmetric band; only bias+tanh+scores are
duplicated per pair orientation (fwd = (k, k+d), rev = (k+d, k)).  The final
band -> [E, E] grid reorder is pure indexing done on the host.

Per-head products go to band12[h]; an add tree folds the 12 heads.  Odd
diagonals read from a host-supplied one-element-shifted copy of the padded
attention rows so every window is 4-byte aligned (bf16 2x mode needs that).
"""

import numpy as np

B, Q, L, H, E, M, R, P, NH = 2, 4, 1024, 768, 32, 2, 5, 10, 12
NCORES = 8
LT = L // 128          # 8 l-tiles
HT = H // 128          # 6 tiles of 128 along a hidden dim
RP = R * P             # 50 prototype rows
ND = 16                # main band diagonals (d = 0..15)
NB = ND * E            # 512 main band slots (= one fp32 PSUM bank)
NS = NB + E            # 544 slots incl. the distance-16 strip
OUTROWS = 1152         # 512 fwd + 512 rev + 32 strip + 96 pad (9 * 128)

_CACHE = {}


def _build_program():
    import concourse.mybir as mybir
    import concourse.tile as tile
    from concourse import bacc

    f32 = mybir.dt.float32
    bf16 = mybir.dt.bfloat16
    nc = bacc.Bacc("TRN2", target_bir_lowering=False, debug=False,
                   num_devices=NCORES)

    atp_d = nc.dram_tensor("atp", [L, NH * 48], bf16, kind="ExternalInput").ap()
    ato_d = nc.dram_tensor("ato", [L, NH * 48], bf16, kind="ExternalInput").ap()
    seq_d = nc.dram_tensor("seq", [L, H], bf16, kind="ExternalInput").ap()
    entT_d = nc.dram_tensor("entT", [H, E], bf16, kind="ExternalInput").ap()
    wh_d = nc.dram_tensor("wh", [2 * H, H], bf16, kind="ExternalInput").ap()
    wt_d = nc.dram_tensor("wt", [2 * H, H], bf16, kind="ExternalInput").ap()
    ptT_d = nc.dram_tensor("ptT", [2 * H, RP], bf16, kind="ExternalInput").ap()
    out_d = nc.dram_tensor("out", [OUTROWS, R], f32, kind="ExternalOutput").ap()

    with tile.TileContext(nc) as tc:
        _emit(tc, mybir, atp_d, ato_d, seq_d, entT_d, wh_d, wt_d, ptT_d, out_d)

    nc.compile()
    return nc


def _win(ap, dims):
    """Replace the trailing free dim of `ap` with explicit [stride, size]
    dims (overlapping windows allowed)."""
    a = ap.copy()
    a.ap = a.ap[:-1] + [list(d) for d in dims]
    return a


def _emit(tc, mybir, atp_d, ato_d, seq_d, entT_d, wh_d, wt_d, ptT_d, out_d):
    nc = tc.nc
    f32 = mybir.dt.float32
    bf16 = mybir.dt.bfloat16
    Alu = mybir.AluOpType
    Act = mybir.ActivationFunctionType
    Ax = mybir.AxisListType
    from concourse.masks import make_identity

    import contextlib
    ctx = contextlib.ExitStack()
    with ctx:
        const = ctx.enter_context(tc.tile_pool(name="const", bufs=1))
        big = ctx.enter_context(tc.tile_pool(name="big", bufs=1))
        bandp = ctx.enter_context(tc.tile_pool(name="bandp", bufs=2))
        mulp = ctx.enter_context(tc.tile_pool(name="mulp", bufs=2))
        candp = ctx.enter_context(tc.tile_pool(name="candp", bufs=13))
        tmp = ctx.enter_context(tc.tile_pool(name="tmp", bufs=4))
        psum = ctx.enter_context(tc.tile_pool(name="psum", bufs=1, space="PSUM"))

        # ---------------- input loads ----------------
        atp_sb = big.tile([128, LT, NH, 48], bf16, tag="atp_sb")
        atp_r = atp_d.rearrange("(t p) (h j) -> p t h j", p=128, h=NH)
        ato_sb = big.tile([128, LT, NH, 48], bf16, tag="ato_sb")
        ato_r = ato_d.rearrange("(t p) (h j) -> p t h j", p=128, h=NH)
        nc.sync.dma_start(out=atp_sb[:, :2], in_=atp_r[:, :2])
        nc.sync.dma_start(out=ato_sb[:, :2], in_=ato_r[:, :2])
        nc.sync.dma_start(out=atp_sb[:, 2:], in_=atp_r[:, 2:])
        nc.sync.dma_start(out=ato_sb[:, 2:], in_=ato_r[:, 2:])
        seq_sb = big.tile([128, LT, H], bf16, tag="seq_sb")
        nc.sync.dma_start(out=seq_sb, in_=seq_d.rearrange("(t p) n -> p t n", p=128))
        entT_sb = const.tile([128, HT, E], bf16, tag="entT_sb")
        nc.sync.dma_start(out=entT_sb, in_=entT_d.rearrange("(t p) n -> p t n", p=128))
        wh_sb = big.tile([128, 2 * HT, H], bf16, tag="wh_sb")
        nc.sync.dma_start(out=wh_sb, in_=wh_d.rearrange("(t p) n -> p t n", p=128))
        wt_sb = big.tile([128, 2 * HT, H], bf16, tag="wt_sb")
        nc.sync.dma_start(out=wt_sb, in_=wt_d.rearrange("(t p) n -> p t n", p=128))
        ptT_sb = const.tile([128, 2 * HT, RP], bf16, tag="ptT_sb")
        nc.sync.dma_start(out=ptT_sb, in_=ptT_d.rearrange("(t p) n -> p t n", p=128))

        ones_col = const.tile([128, 1], bf16, tag="ones_col")
        nc.vector.memset(ones_col, 1.0)
        ones_row = const.tile([1, 128], f32, tag="ones_row")
        nc.vector.memset(ones_row, 1.0)
        ident = const.tile([RP, RP], f32, tag="ident")
        make_identity(nc, ident)

        # ---------------- band products + fold + ctx/S accumulate ----------
        ctx_ps = [psum.tile([128, NB], f32, tag="ctx", bufs=HT,
                            name=f"ctx{ht}") for ht in range(HT)]
        # one aux bank: ctx-strip [6*32] at cols 0:192, S-strip at 192:224,
        # ep scratch at 224:256
        ctxs_ps = psum.tile([128, 256], f32, tag="ctxs", bufs=1, name="ctxs")
        s_ps = psum.tile([1, NB], f32, tag="s", bufs=1, name="s_ps")
        ep_sb = const.tile([128, 2, HT, 48], bf16, tag="ep_sb")

        def emit_ep():
            # entity projections ep[w][h'', e] = W[:H].T @ entT (interleaved
            # mid-loop: after wh/wt have landed, before PE runs dry)
            for w, wsb in ((0, wh_sb), (1, wt_sb)):
                for ht2 in range(HT):
                    ps = ctxs_ps[:, 224:256]
                    for kt in range(HT):
                        nc.tensor.matmul(
                            ps, wsb[:, kt, ht2 * 128:(ht2 + 1) * 128],
                            entT_sb[:, kt, :],
                            start=(kt == 0), stop=(kt == HT - 1))
                    nc.scalar.copy(ep_sb[:, w, ht2, :E], ps)
                    nc.scalar.copy(ep_sb[:, w, ht2, E:], ep_sb[:, w, ht2, :16])

        for lt in range(LT):
            xp = atp_sb[:, lt]                    # [128, NH, 48]
            xo = ato_sb[:, lt]
            b12 = bandp.tile([128, NH, NS], bf16, tag="b12", name=f"b12_{lt}")
            # in0: X[k] broadcast over 8 diagonals (outer stride 0, inner 1)
            x_b = xp[:, :, None, :E].broadcast_to([128, NH, 8, E])
            # even diagonals d = 0,2,..,14: in1 = X[k+d] (window stride 2)
            nc.vector.tensor_tensor(
                out=_win(b12, [[2 * E, 8], [1, E]]),
                in0=x_b, in1=_win(xp, [[2, 8], [1, E]]),
                op=Alu.mult)
            # odd diagonals d = 1,3,..,15: in1 = Xodd[k+d-1], Xodd[j]=X[j+1]
            nc.vector.tensor_tensor(
                out=_win(b12[:, :, E:], [[2 * E, 8], [1, E]]),
                in0=x_b, in1=_win(xo, [[2, 8], [1, E]]),
                op=Alu.mult)
            # strip d = 16: in1 = X[k+16]
            nc.vector.tensor_tensor(
                out=b12[:, :, NB:], in0=xp[:, :, :E], in1=xp[:, :, 16:],
                op=Alu.mult)
            # fold 12 heads -> band_mul
            nc.vector.tensor_add(b12[:, 0:6], b12[:, 0:6], b12[:, 6:12])
            nc.vector.tensor_add(b12[:, 0:3], b12[:, 0:3], b12[:, 3:6])
            nc.vector.tensor_add(b12[:, 0], b12[:, 0], b12[:, 1])
            bm = mulp.tile([128, NS], bf16, tag="bm", name=f"bm_{lt}")
            nc.vector.tensor_add(bm, b12[:, 0], b12[:, 2])

            # ctx += seq_lt.T @ band_mul ; S += ones.T @ band_mul
            st = (lt == 0)
            sp = (lt == LT - 1)
            for ht in range(HT):
                stat = seq_sb[:, lt, ht * 128:(ht + 1) * 128]
                nc.tensor.matmul(ctx_ps[ht], stat, bm[:, :NB],
                                 start=st, stop=sp)
                nc.tensor.matmul(ctxs_ps[:, ht * E:(ht + 1) * E], stat,
                                 bm[:, NB:], start=st, stop=sp)
            nc.tensor.matmul(s_ps, ones_col, bm[:, :NB], start=st, stop=sp)
            nc.tensor.matmul(ctxs_ps[0:1, 192:224], ones_col, bm[:, NB:],
                             start=st, stop=sp)
            if lt == 3:
                emit_ep()

        # ---------------- 1/S broadcast to all partitions ----------------
        s_sb = const.tile([1, NS], f32, tag="s_sb")
        nc.scalar.copy(s_sb[:, :NB], s_ps)
        nc.scalar.copy(s_sb[:, NB:], ctxs_ps[0:1, 192:224])
        nc.vector.reciprocal_approx_fast(out=s_sb, in_=s_sb)
        recS_sb = const.tile([128, NS], f32, tag="recS_sb")
        rb = psum.tile([128, NB], f32, tag="s", bufs=1, name="recB")
        nc.tensor.matmul(rb, ones_row, s_sb[:, :NB], start=True, stop=True)
        nc.scalar.copy(recS_sb[:, :NB], rb)
        rb2 = psum.tile([128, E], f32, tag="s", bufs=1, name="recB2")
        nc.tensor.matmul(rb2, ones_row, s_sb[:, NB:], start=True, stop=True)
        nc.scalar.copy(recS_sb[:, NB:], rb2)

        # ---------------- normalize: cn = ctx * (1/S), bf16 ----------------
        cn = big.tile([128, HT, NS], bf16, tag="cn")
        for ht in range(HT):
            nc.vector.tensor_mul(cn[:, ht, :NB], ctx_ps[ht],
                                 recS_sb[:, :NB])
        nc.vector.tensor_mul(
            cn[:, :, NB:],
            ctxs_ps[:, 0:192].rearrange("p (a b) -> p a b", a=HT),
            recS_sb[:, None, NB:].broadcast_to([128, HT, E]))

        # ---------------- proj + bias + tanh (both orientations) -----------
        candF = [None] * (2 * HT)
        candR = [None] * (2 * HT)
        for g in range(2 * HT):
            w, ht2 = divmod(g, HT)
            wsb = wh_sb if w == 0 else wt_sb
            ps = psum.tile([128, NB], f32, tag="ctx", bufs=HT,
                           name=f"proj{g}")
            pss = psum.tile([128, E], f32, tag="ctx", bufs=HT,
                            name=f"projs{g}")
            for kt in range(HT):
                stat = wsb[:, HT + kt, ht2 * 128:(ht2 + 1) * 128]
                nc.tensor.matmul(ps, stat, cn[:, kt, :NB],
                                 start=(kt == 0), stop=(kt == HT - 1))
                nc.tensor.matmul(pss, stat, cn[:, kt, NB:],
                                 start=(kt == 0), stop=(kt == HT - 1))
            epw = ep_sb[:, w, ht2]
            # fwd pair (k, k+d): head bias = ep[k], tail bias = ep[k+d]
            if w == 0:
                bias_f = _win(epw, [[0, ND], [1, E]])
                bias_fs = epw[:, :E]
                bias_r = _win(epw, [[1, ND], [1, E]])
            else:
                bias_f = _win(epw, [[1, ND], [1, E]])
                bias_fs = epw[:, 16:]
                bias_r = _win(epw, [[0, ND], [1, E]])
            pre_f = tmp.tile([128, NS], bf16, tag="pre", name=f"pref{g}")
            nc.vector.tensor_add(
                pre_f[:, :NB].rearrange("p (a b) -> p a b", a=ND),
                ps.rearrange("p (a b) -> p a b", a=ND), bias_f)
            nc.vector.tensor_add(pre_f[:, NB:], pss, bias_fs)
            cf = candp.tile([128, NS], bf16, tag="candF", name=f"candF{g}")
            candF[g] = cf
            nc.scalar.activation(cf, pre_f, Act.Tanh)

            pre_r = tmp.tile([128, NB], bf16, tag="pre", name=f"prer{g}")
            nc.vector.tensor_add(
                pre_r.rearrange("p (a b) -> p a b", a=ND),
                ps.rearrange("p (a b) -> p a b", a=ND), bias_r)
            cr = candp.tile([128, NB], bf16, tag="candR", name=f"candR{g}")
            candR[g] = cr
            nc.scalar.activation(cr, pre_r, Act.Tanh)

        # ------- scores + per-class max, pipelined per 128-slot chunk -------
        obF = const.tile([128, 4, R], f32, tag="obF")
        obR = const.tile([128, 4, R], f32, tag="obR")
        obS = const.tile([E, R], f32, tag="obS")
        scT = const.tile([RP, 9, 128], f32, tag="scT")
        for c in range(9):
            wdt = E if c == 8 else 128
            sc = psum.tile([RP, wdt], f32, tag="ctx", bufs=HT, name=f"sc{c}")
            for g in range(2 * HT):
                if c == 8:
                    mv = candF[g][:, NB:]
                elif c < 4:
                    mv = candF[g][:, c * 128:(c + 1) * 128]
                else:
                    mv = candR[g][:, (c - 4) * 128:(c - 3) * 128]
                nc.tensor.matmul(sc, ptT_sb[:, g, :], mv,
                                 start=(g == 0), stop=(g == 2 * HT - 1))
            nc.scalar.copy(scT[:, c, :wdt], sc)
            tp = psum.tile([wdt, RP], f32, tag="ctx", bufs=HT, name=f"tp{c}")
            nc.tensor.transpose(tp, scT[:, c, :wdt], ident)
            if c == 8:
                dst = obS[:, None, :]
            elif c < 4:
                dst = obF[:, c, None, :]
            else:
                dst = obR[:, c - 4, None, :]
            nc.vector.tensor_reduce(
                out=dst.rearrange("p a b -> p (a b)"),
                in_=tp.rearrange("p (r q) -> p r q", r=R),
                axis=Ax.X, op=Alu.max)

        out_r = out_d.rearrange("(c p) r -> p c r", p=128)
        nc.scalar.dma_start(out=out_r[:, 0:4], in_=obF)
        nc.scalar.dma_start(out=out_r[:, 4:8], in_=obR)
        nc.scalar.dma_start(out=out_d[1024:1056].rearrange(
            "(a p) r -> p a r", p=E), in_=obS[:, None, :])


def _host_prep(sequence_output, attention, W_head, W_tail, prototypes,
               mention_pos):
    """Build the per-core input maps (numpy indexing / dtype conversion)."""
    import ml_dtypes
    bf = ml_dtypes.bfloat16

    seq = np.asarray(sequence_output, dtype=np.float32)
    att = np.asarray(attention, dtype=np.float32)
    wh = np.ascontiguousarray(W_head, dtype=np.float32).astype(bf)
    wt = np.ascontiguousarray(W_tail, dtype=np.float32).astype(bf)
    pro = np.asarray(prototypes, dtype=np.float32)
    pos = np.asarray(mention_pos)

    in_maps = []
    for c in range(NCORES):
        b, q = divmod(c, Q)
        p_bq = pos[b, q]                       # [E, M]
        # attention gather + mention-sum: At[l, h, e]  (scale dropped)
        g = att[b, q][:, p_bq, :]              # [NH, E, M, L]
        asum = (g[:, :, 0, :] + g[:, :, 1, :])  # [NH, E, L]
        at = np.ascontiguousarray(np.transpose(asum, (2, 0, 1)))  # [L,NH,E]
        atp = np.concatenate([at, at[:, :, :16]], axis=2)  # [L, NH, 48]
        ato = np.zeros_like(atp)
        ato[:, :, :47] = atp[:, :, 1:]
        # entity means: ent[e] = mean_m seq[pos]  -> entT [H, E]
        ment = seq[b, q][p_bq]                 # [E, M, H]
        ent = (ment[:, 0, :] + ment[:, 1, :]) * np.float32(0.5)
        entT = np.ascontiguousarray(ent.T)
        ptT = np.ascontiguousarray(pro[b].reshape(RP, 2 * H).T)
        in_maps.append({
            "atp": atp.reshape(L, NH * 48).astype(bf),
            "ato": ato.reshape(L, NH * 48).astype(bf),
            "seq": seq[b, q].astype(bf),
            "entT": entT.astype(bf),
            "wh": wh,
            "wt": wt,
            "ptT": ptT.astype(bf),
        })
    return in_maps


# band slot -> grid index maps (precomputed once)
def _unband_index():
    dd, kk = np.meshgrid(np.arange(ND), np.arange(E), indexing="ij")
    f_fwd = (kk + dd) % E
    # fwd slot (d, k) -> (k, (k+d)%32); rev slot -> ((k+d)%32, k)
    fwd_e, fwd_f = kk.ravel(), f_fwd.ravel()
    rev_e, rev_f = f_fwd.ravel(), kk.ravel()
    k = np.arange(E)
    strip_e, strip_f = k, (k + 16) % E
    return fwd_e, fwd_f, rev_e, rev_f, strip_e, strip_f


def kernel(sequence_output, attention, W_head, W_tail, prototypes,
           mention_pos):
    from concourse.bass_utils import run_bass_kernel_spmd

    if "nc" not in _CACHE:
        _CACHE["nc"] = _build_program()
        _CACHE["idx"] = _unband_index()
    nc = _CACHE["nc"]
    fwd_e, fwd_f, rev_e, rev_f, strip_e, strip_f = _CACHE["idx"]

    in_maps = _host_prep(sequence_output, attention, W_head, W_tail,
                         prototypes, mention_pos)
    res = run_bass_kernel_spmd(nc, in_maps, core_ids=list(range(NCORES)))

    out = np.empty((B, Q, E, E, R), dtype=np.float32)
    for c in range(NCORES):
        b, q = divmod(c, Q)
        raw = res.results[c]["out"]
        grid = out[b, q]
        grid[rev_e, rev_f] = raw[512:1024]
        grid[fwd_e, fwd_f] = raw[0:512]
        grid[strip_e, strip_f] = raw[1024:1056]
    return out


# revision 22
# speedup vs baseline: 1.0590x; 1.0590x over previous
"""Trainium2 Bass kernel for nn_BaseEncoder (ragged entity-pair encoder).

Contract: kernel(**inputs) takes the FULL unsharded inputs (numpy) and
returns the FULL output [B, Q, E, E, R] float32.

Sharding: B*Q = 8 independent (batch, query) pairs -> one per NeuronCore.

Diagonal-band formulation (v2): the pair tensor mul[l, e, f] is symmetric in
(e, f), so the device computes it only on 544 unique "band" slots:
  main band: slot (d, k), d in 0..15, k in 0..31  -> pair (k, (k+d) % 32)
  strip:     slot k,      k in 0..31              -> pair (k, (k+16) % 32)
Band products use overlapping-window / outer-broadcast access patterns whose
inner stride is 1, which keeps the DVE in its 2x bf16 perf mode (the previous
grid formulation's inner-stride-0 broadcasts ran at 1x).  ctx / S / norm /
proj all run once on the shared symmetric band; only bias+tanh+scores are
duplicated per pair orientation (fwd = (k, k+d), rev = (k+d, k)).  The final
band -> [E, E] grid reorder is pure indexing done on the host.

Per-head products go to band12[h]; an add tree folds the 12 heads.  Odd
diagonals read from a host-supplied one-element-shifted copy of the padded
attention rows so every window is 4-byte aligned (bf16 2x mode needs that).
"""

import numpy as np

B, Q, L, H, E, M, R, P, NH = 2, 4, 1024, 768, 32, 2, 5, 10, 12
NCORES = 8
LT = L // 128          # 8 l-tiles
HT = H // 128          # 6 tiles of 128 along a hidden dim
RP = R * P             # 50 prototype rows
ND = 16                # main band diagonals (d = 0..15)
NB = ND * E            # 512 main band slots (= one fp32 PSUM bank)
NS = NB + E            # 544 slots incl. the distance-16 strip
OUTROWS = 1152         # 512 fwd + 512 rev + 32 strip + 96 pad (9 * 128)

_CACHE = {}


def _build_program():
    import concourse.mybir as mybir
    import concourse.tile as tile
    from concourse import bacc

    f32 = mybir.dt.float32
    bf16 = mybir.dt.bfloat16
    nc = bacc.Bacc("TRN2", target_bir_lowering=False, debug=False,
                   num_devices=NCORES)

    atp_d = nc.dram_tensor("atp", [L, NH * 48], bf16, kind="ExternalInput").ap()
    ato_d = nc.dram_tensor("ato", [L, NH * 48], bf16, kind="ExternalInput").ap()
    seq_d = nc.dram_tensor("seq", [L, H], bf16, kind="ExternalInput").ap()
    entT_d = nc.dram_tensor("entT", [H, E], bf16, kind="ExternalInput").ap()
    wh_d = nc.dram_tensor("wh", [2 * H, H], bf16, kind="ExternalInput").ap()
    wt_d = nc.dram_tensor("wt", [2 * H, H], bf16, kind="ExternalInput").ap()
    ptT_d = nc.dram_tensor("ptT", [2 * H, RP], bf16, kind="ExternalInput").ap()
    out_d = nc.dram_tensor("out", [OUTROWS, R], f32, kind="ExternalOutput").ap()

    with tile.TileContext(nc) as tc:
        _emit(tc, mybir, atp_d, ato_d, seq_d, entT_d, wh_d, wt_d, ptT_d, out_d)

    nc.compile()
    return nc


def _win(ap, dims):
    """Replace the trailing free dim of `ap` with explicit [stride, size]
    dims (overlapping windows allowed)."""
    a = ap.copy()
    a.ap = a.ap[:-1] + [list(d) for d in dims]
    return a


def _emit(tc, mybir, atp_d, ato_d, seq_d, entT_d, wh_d, wt_d, ptT_d, out_d):
    nc = tc.nc
    f32 = mybir.dt.float32
    bf16 = mybir.dt.bfloat16
    Alu = mybir.AluOpType
    Act = mybir.ActivationFunctionType
    Ax = mybir.AxisListType
    from concourse.masks import make_identity

    import contextlib
    ctx = contextlib.ExitStack()
    with ctx:
        const = ctx.enter_context(tc.tile_pool(name="const", bufs=1))
        big = ctx.enter_context(tc.tile_pool(name="big", bufs=1))
        bandp = ctx.enter_context(tc.tile_pool(name="bandp", bufs=2))
        mulp = ctx.enter_context(tc.tile_pool(name="mulp", bufs=2))
        candp = ctx.enter_context(tc.tile_pool(name="candp", bufs=13))
        tmp = ctx.enter_context(tc.tile_pool(name="tmp", bufs=4))
        psum = ctx.enter_context(tc.tile_pool(name="psum", bufs=1, space="PSUM"))

        # ---------------- input loads ----------------
        atp_sb = big.tile([128, LT, NH, 48], bf16, tag="atp_sb")
        atp_r = atp_d.rearrange("(t p) (h j) -> p t h j", p=128, h=NH)
        ato_sb = big.tile([128, LT, NH, 48], bf16, tag="ato_sb")
        ato_r = ato_d.rearrange("(t p) (h j) -> p t h j", p=128, h=NH)
        nc.sync.dma_start(out=atp_sb[:, :2], in_=atp_r[:, :2])
        nc.sync.dma_start(out=ato_sb[:, :2], in_=ato_r[:, :2])
        nc.sync.dma_start(out=atp_sb[:, 2:], in_=atp_r[:, 2:])
        nc.sync.dma_start(out=ato_sb[:, 2:], in_=ato_r[:, 2:])
        seq_sb = big.tile([128, LT, H], bf16, tag="seq_sb")
        nc.sync.dma_start(out=seq_sb, in_=seq_d.rearrange("(t p) n -> p t n", p=128))
        entT_sb = const.tile([128, HT, E], bf16, tag="entT_sb")
        nc.sync.dma_start(out=entT_sb, in_=entT_d.rearrange("(t p) n -> p t n", p=128))
        wh_sb = big.tile([128, 2 * HT, H], bf16, tag="wh_sb")
        nc.sync.dma_start(out=wh_sb, in_=wh_d.rearrange("(t p) n -> p t n", p=128))
        wt_sb = big.tile([128, 2 * HT, H], bf16, tag="wt_sb")
        nc.sync.dma_start(out=wt_sb, in_=wt_d.rearrange("(t p) n -> p t n", p=128))
        ptT_sb = const.tile([128, 2 * HT, RP], bf16, tag="ptT_sb")
        nc.sync.dma_start(out=ptT_sb, in_=ptT_d.rearrange("(t p) n -> p t n", p=128))

        ones_col = const.tile([128, 1], bf16, tag="ones_col")
        nc.vector.memset(ones_col, 1.0)
        ones_row = const.tile([1, 128], f32, tag="ones_row")
        nc.vector.memset(ones_row, 1.0)
        ident = const.tile([RP, RP], f32, tag="ident")
        make_identity(nc, ident)

        # ---------------- band products + fold + ctx/S accumulate ----------
        ctx_ps = [psum.tile([128, NB], f32, tag="ctx", bufs=HT,
                            name=f"ctx{ht}") for ht in range(HT)]
        # one aux bank: ctx-strip [6*32] at cols 0:192, S-strip at 192:224,
        # ep scratch at 224:256
        ctxs_ps = psum.tile([128, 256], f32, tag="ctxs", bufs=1, name="ctxs")
        s_ps = psum.tile([1, NB], f32, tag="s", bufs=1, name="s_ps")
        ep_sb = const.tile([128, 2, HT, 48], bf16, tag="ep_sb")

        def emit_ep():
            # entity projections ep[w][h'', e] = W[:H].T @ entT (interleaved
            # mid-loop: after wh/wt have landed, before PE runs dry)
            for w, wsb in ((0, wh_sb), (1, wt_sb)):
                for ht2 in range(HT):
                    ps = ctxs_ps[:, 224:256]
                    for kt in range(HT):
                        nc.tensor.matmul(
                            ps, wsb[:, kt, ht2 * 128:(ht2 + 1) * 128],
                            entT_sb[:, kt, :],
                            start=(kt == 0), stop=(kt == HT - 1))
                    nc.scalar.copy(ep_sb[:, w, ht2, :E], ps)
                    nc.scalar.copy(ep_sb[:, w, ht2, E:], ep_sb[:, w, ht2, :16])

        for lt in range(LT):
            xp = atp_sb[:, lt]                    # [128, NH, 48]
            xo = ato_sb[:, lt]
            b12 = bandp.tile([128, NH, NS], bf16, tag="b12", name=f"b12_{lt}")
            # in0: X[k] broadcast over 8 diagonals (outer stride 0, inner 1)
            x_b = xp[:, :, None, :E].broadcast_to([128, NH, 8, E])
            # even diagonals d = 0,2,..,14: in1 = X[k+d] (window stride 2)
            nc.vector.tensor_tensor(
                out=_win(b12, [[2 * E, 8], [1, E]]),
                in0=x_b, in1=_win(xp, [[2, 8], [1, E]]),
                op=Alu.mult)
            # odd diagonals d = 1,3,..,15: in1 = Xodd[k+d-1], Xodd[j]=X[j+1]
            nc.vector.tensor_tensor(
                out=_win(b12[:, :, E:], [[2 * E, 8], [1, E]]),
                in0=x_b, in1=_win(xo, [[2, 8], [1, E]]),
                op=Alu.mult)
            # strip d = 16: in1 = X[k+16]
            nc.vector.tensor_tensor(
                out=b12[:, :, NB:], in0=xp[:, :, :E], in1=xp[:, :, 16:],
                op=Alu.mult)
            # fold 12 heads -> band_mul
            nc.vector.tensor_add(b12[:, 0:6], b12[:, 0:6], b12[:, 6:12])
            nc.vector.tensor_add(b12[:, 0:3], b12[:, 0:3], b12[:, 3:6])
            nc.vector.tensor_add(b12[:, 0], b12[:, 0], b12[:, 1])
            bm = mulp.tile([128, NS], bf16, tag="bm", name=f"bm_{lt}")
            nc.vector.tensor_add(bm, b12[:, 0], b12[:, 2])

            # ctx += seq_lt.T @ band_mul ; S += ones.T @ band_mul
            st = (lt == 0)
            sp = (lt == LT - 1)
            for ht in range(HT):
                stat = seq_sb[:, lt, ht * 128:(ht + 1) * 128]
                nc.tensor.matmul(ctx_ps[ht], stat, bm[:, :NB],
                                 start=st, stop=sp)
                nc.tensor.matmul(ctxs_ps[:, ht * E:(ht + 1) * E], stat,
                                 bm[:, NB:], start=st, stop=sp)
            nc.tensor.matmul(s_ps, ones_col, bm[:, :NB], start=st, stop=sp)
            nc.tensor.matmul(ctxs_ps[0:1, 192:224], ones_col, bm[:, NB:],
                             start=st, stop=sp)
            if lt == 3:
                emit_ep()

        # ---------------- 1/S broadcast to all partitions ----------------
        s_sb = const.tile([1, NS], f32, tag="s_sb")
        nc.scalar.copy(s_sb[:, :NB], s_ps)
        nc.scalar.copy(s_sb[:, NB:], ctxs_ps[0:1, 192:224])
        nc.vector.reciprocal_approx_fast(out=s_sb, in_=s_sb)
        recS_sb = const.tile([128, NS], f32, tag="recS_sb")
        rb = psum.tile([128, NB], f32, tag="s", bufs=1, name="recB")
        nc.tensor.matmul(rb, ones_row, s_sb[:, :NB], start=True, stop=True)
        nc.scalar.copy(recS_sb[:, :NB], rb)
        rb2 = psum.tile([128, E], f32, tag="s", bufs=1, name="recB2")
        nc.tensor.matmul(rb2, ones_row, s_sb[:, NB:], start=True, stop=True)
        nc.scalar.copy(recS_sb[:, NB:], rb2)

        # ---------------- normalize: cn = ctx * (1/S), bf16 ----------------
        cn = big.tile([128, HT, NS], bf16, tag="cn")
        for ht in range(HT):
            nc.vector.tensor_mul(cn[:, ht, :NB], ctx_ps[ht],
                                 recS_sb[:, :NB])
        nc.vector.tensor_mul(
            cn[:, :, NB:],
            ctxs_ps[:, 0:192].rearrange("p (a b) -> p a b", a=HT),
            recS_sb[:, None, NB:].broadcast_to([128, HT, E]))

        # ---------------- proj + bias + tanh (both orientations) -----------
        candF = [None] * (2 * HT)
        candR = [None] * (2 * HT)
        for g in range(2 * HT):
            w, ht2 = divmod(g, HT)
            wsb = wh_sb if w == 0 else wt_sb
            ps = psum.tile([128, NB], f32, tag="ctx", bufs=HT,
                           name=f"proj{g}")
            pss = psum.tile([128, E], f32, tag="ctx", bufs=HT,
                            name=f"projs{g}")
            for kt in range(HT):
                stat = wsb[:, HT + kt, ht2 * 128:(ht2 + 1) * 128]
                nc.tensor.matmul(ps, stat, cn[:, kt, :NB],
                                 start=(kt == 0), stop=(kt == HT - 1))
                nc.tensor.matmul(pss, stat, cn[:, kt, NB:],
                                 start=(kt == 0), stop=(kt == HT - 1))
            pj = tmp.tile([128, NS], bf16, tag="pj", name=f"pj{g}")
            nc.scalar.copy(pj[:, :NB], ps)
            nc.scalar.copy(pj[:, NB:], pss)

            epw = ep_sb[:, w, ht2]
            # fwd pair (k, k+d): head bias = ep[k], tail bias = ep[k+d]
            # (strip row d=16 included -> 17 "diagonals")
            if w == 0:
                bias_f = _win(epw, [[0, ND + 1], [1, E]])
                bias_r = _win(epw, [[1, ND], [1, E]])
            else:
                bias_f = _win(epw, [[1, ND + 1], [1, E]])
                bias_r = _win(epw, [[0, ND], [1, E]])
            pre_f = tmp.tile([128, NS], bf16, tag="pre", name=f"pref{g}")
            nc.vector.tensor_add(
                pre_f.rearrange("p (a b) -> p a b", a=ND + 1),
                pj.rearrange("p (a b) -> p a b", a=ND + 1), bias_f)
            cf = candp.tile([128, NS], bf16, tag="candF", name=f"candF{g}")
            candF[g] = cf
            nc.scalar.activation(cf, pre_f, Act.Tanh)

            pre_r = tmp.tile([128, NB], bf16, tag="pre", name=f"prer{g}")
            nc.vector.tensor_add(
                pre_r.rearrange("p (a b) -> p a b", a=ND),
                pj[:, :NB].rearrange("p (a b) -> p a b", a=ND), bias_r)
            cr = candp.tile([128, NB], bf16, tag="candR", name=f"candR{g}")
            candR[g] = cr
            nc.scalar.activation(cr, pre_r, Act.Tanh)

        # ---------------- scores + per-class max ----------------
        scf_ps = psum.tile([RP, NB], f32, tag="s", bufs=1, name="scf")
        scs_ps = psum.tile([RP, E], f32, tag="ctxs", bufs=1, name="scs")
        scr_ps = psum.tile([RP, NB], f32, tag="ctx", bufs=HT, name="scr")
        for g in range(2 * HT):
            st = (g == 0)
            sp = (g == 2 * HT - 1)
            nc.tensor.matmul(scf_ps, ptT_sb[:, g, :], candF[g][:, :NB],
                             start=st, stop=sp)
            nc.tensor.matmul(scs_ps, ptT_sb[:, g, :], candF[g][:, NB:],
                             start=st, stop=sp)
            nc.tensor.matmul(scr_ps, ptT_sb[:, g, :], candR[g],
                             start=st, stop=sp)
        scT = const.tile([RP, 2 * NB + E], f32, tag="scT")
        nc.scalar.copy(scT[:, :NB], scf_ps)
        nc.scalar.copy(scT[:, NB:2 * NB], scr_ps)
        nc.scalar.copy(scT[:, 2 * NB:], scs_ps)

        obF = const.tile([128, 4, R], f32, tag="obF")
        obR = const.tile([128, 4, R], f32, tag="obR")
        obS = const.tile([E, R], f32, tag="obS")
        for c in range(9):
            wdt = E if c == 8 else 128
            tp = psum.tile([wdt, RP], f32, tag="ctx", bufs=HT, name=f"tp{c}")
            nc.tensor.transpose(tp, scT[:, c * 128:c * 128 + wdt], ident)
            if c == 8:
                dst = obS[:, None, :]
            elif c < 4:
                dst = obF[:, c, None, :]
            else:
                dst = obR[:, c - 4, None, :]
            nc.vector.tensor_reduce(
                out=dst.rearrange("p a b -> p (a b)"),
                in_=tp.rearrange("p (r q) -> p r q", r=R),
                axis=Ax.X, op=Alu.max)

        out_r = out_d.rearrange("(c p) r -> p c r", p=128)
        nc.scalar.dma_start(out=out_r[:, 0:4], in_=obF)
        nc.scalar.dma_start(out=out_r[:, 4:8], in_=obR)
        nc.scalar.dma_start(out=out_d[1024:1056].rearrange(
            "(a p) r -> p a r", p=E), in_=obS[:, None, :])


def _host_prep(sequence_output, attention, W_head, W_tail, prototypes,
               mention_pos):
    """Build the per-core input maps (numpy indexing / dtype conversion)."""
    import ml_dtypes
    bf = ml_dtypes.bfloat16

    seq = np.asarray(sequence_output, dtype=np.float32)
    att = np.asarray(attention, dtype=np.float32)
    wh = np.ascontiguousarray(W_head, dtype=np.float32).astype(bf)
    wt = np.ascontiguousarray(W_tail, dtype=np.float32).astype(bf)
    pro = np.asarray(prototypes, dtype=np.float32)
    pos = np.asarray(mention_pos)

    in_maps = []
    for c in range(NCORES):
        b, q = divmod(c, Q)
        p_bq = pos[b, q]                       # [E, M]
        # attention gather + mention-sum: At[l, h, e]  (scale dropped)
        g = att[b, q][:, p_bq, :]              # [NH, E, M, L]
        asum = (g[:, :, 0, :] + g[:, :, 1, :])  # [NH, E, L]
        at = np.ascontiguousarray(np.transpose(asum, (2, 0, 1)))  # [L,NH,E]
        atp = np.concatenate([at, at[:, :, :16]], axis=2)  # [L, NH, 48]
        ato = np.zeros_like(atp)
        ato[:, :, :47] = atp[:, :, 1:]
        # entity means: ent[e] = mean_m seq[pos]  -> entT [H, E]
        ment = seq[b, q][p_bq]                 # [E, M, H]
        ent = (ment[:, 0, :] + ment[:, 1, :]) * np.float32(0.5)
        entT = np.ascontiguousarray(ent.T)
        ptT = np.ascontiguousarray(pro[b].reshape(RP, 2 * H).T)
        in_maps.append({
            "atp": atp.reshape(L, NH * 48).astype(bf),
            "ato": ato.reshape(L, NH * 48).astype(bf),
            "seq": seq[b, q].astype(bf),
            "entT": entT.astype(bf),
            "wh": wh,
            "wt": wt,
            "ptT": ptT.astype(bf),
        })
    return in_maps


# band slot -> grid index maps (precomputed once)
def _unband_index():
    dd, kk = np.meshgrid(np.arange(ND), np.arange(E), indexing="ij")
    f_fwd = (kk + dd) % E
    # fwd slot (d, k) -> (k, (k+d)%32); rev slot -> ((k+d)%32, k)
    fwd_e, fwd_f = kk.ravel(), f_fwd.ravel()
    rev_e, rev_f = f_fwd.ravel(), kk.ravel()
    k = np.arange(E)
    strip_e, strip_f = k, (k + 16) % E
    return fwd_e, fwd_f, rev_e, rev_f, strip_e, strip_f


def kernel(sequence_output, attention, W_head, W_tail, prototypes,
           mention_pos):
    from concourse.bass_utils import run_bass_kernel_spmd

    if "nc" not in _CACHE:
        _CACHE["nc"] = _build_program()
        _CACHE["idx"] = _unband_index()
    nc = _CACHE["nc"]
    fwd_e, fwd_f, rev_e, rev_f, strip_e, strip_f = _CACHE["idx"]

    in_maps = _host_prep(sequence_output, attention, W_head, W_tail,
                         prototypes, mention_pos)
    res = run_bass_kernel_spmd(nc, in_maps, core_ids=list(range(NCORES)))

    out = np.empty((B, Q, E, E, R), dtype=np.float32)
    for c in range(NCORES):
        b, q = divmod(c, Q)
        raw = res.results[c]["out"]
        grid = out[b, q]
        grid[rev_e, rev_f] = raw[512:1024]
        grid[fwd_e, fwd_f] = raw[0:512]
        grid[strip_e, strip_f] = raw[1024:1056]
    return out


# revision 23
# speedup vs baseline: 1.0632x; 1.0039x over previous
"""Trainium2 Bass kernel for nn_BaseEncoder (ragged entity-pair encoder).

Contract: kernel(**inputs) takes the FULL unsharded inputs (numpy) and
returns the FULL output [B, Q, E, E, R] float32.

Sharding: B*Q = 8 independent (batch, query) pairs -> one per NeuronCore.

Diagonal-band formulation (v2): the pair tensor mul[l, e, f] is symmetric in
(e, f), so the device computes it only on 544 unique "band" slots:
  main band: slot (d, k), d in 0..15, k in 0..31  -> pair (k, (k+d) % 32)
  strip:     slot k,      k in 0..31              -> pair (k, (k+16) % 32)
Band products use overlapping-window / outer-broadcast access patterns whose
inner stride is 1, which keeps the DVE in its 2x bf16 perf mode (the previous
grid formulation's inner-stride-0 broadcasts ran at 1x).  ctx / S / norm /
proj all run once on the shared symmetric band; only bias+tanh+scores are
duplicated per pair orientation (fwd = (k, k+d), rev = (k+d, k)).  The final
band -> [E, E] grid reorder is pure indexing done on the host.

Per-head products go to band12[h]; an add tree folds the 12 heads.  Odd
diagonals read from a host-supplied one-element-shifted copy of the padded
attention rows so every window is 4-byte aligned (bf16 2x mode needs that).
"""

import numpy as np

B, Q, L, H, E, M, R, P, NH = 2, 4, 1024, 768, 32, 2, 5, 10, 12
NCORES = 8
LT = L // 128          # 8 l-tiles
HT = H // 128          # 6 tiles of 128 along a hidden dim
RP = R * P             # 50 prototype rows
ND = 16                # main band diagonals (d = 0..15)
NB = ND * E            # 512 main band slots (= one fp32 PSUM bank)
NS = NB + E            # 544 slots incl. the distance-16 strip
OUTROWS = 1152         # 512 fwd + 512 rev + 32 strip + 96 pad (9 * 128)

_CACHE = {}


def _build_program():
    import concourse.mybir as mybir
    import concourse.tile as tile
    from concourse import bacc

    f32 = mybir.dt.float32
    bf16 = mybir.dt.bfloat16
    nc = bacc.Bacc("TRN2", target_bir_lowering=False, debug=False,
                   num_devices=NCORES)

    atp_d = nc.dram_tensor("atp", [L, NH * 48], bf16, kind="ExternalInput").ap()
    ato_d = nc.dram_tensor("ato", [L, NH * 48], bf16, kind="ExternalInput").ap()
    seq_d = nc.dram_tensor("seq", [L, H], bf16, kind="ExternalInput").ap()
    entT_d = nc.dram_tensor("entT", [H, E], bf16, kind="ExternalInput").ap()
    wh_d = nc.dram_tensor("wh", [2 * H, H], bf16, kind="ExternalInput").ap()
    wt_d = nc.dram_tensor("wt", [2 * H, H], bf16, kind="ExternalInput").ap()
    ptT_d = nc.dram_tensor("ptT", [2 * H, RP], bf16, kind="ExternalInput").ap()
    out_d = nc.dram_tensor("out", [OUTROWS, R], f32, kind="ExternalOutput").ap()

    with tile.TileContext(nc) as tc:
        _emit(tc, mybir, atp_d, ato_d, seq_d, entT_d, wh_d, wt_d, ptT_d, out_d)

    nc.compile()
    return nc


def _win(ap, dims):
    """Replace the trailing free dim of `ap` with explicit [stride, size]
    dims (overlapping windows allowed)."""
    a = ap.copy()
    a.ap = a.ap[:-1] + [list(d) for d in dims]
    return a


def _emit(tc, mybir, atp_d, ato_d, seq_d, entT_d, wh_d, wt_d, ptT_d, out_d):
    nc = tc.nc
    f32 = mybir.dt.float32
    bf16 = mybir.dt.bfloat16
    Alu = mybir.AluOpType
    Act = mybir.ActivationFunctionType
    Ax = mybir.AxisListType
    from concourse.masks import make_identity

    import contextlib
    ctx = contextlib.ExitStack()
    with ctx:
        const = ctx.enter_context(tc.tile_pool(name="const", bufs=1))
        big = ctx.enter_context(tc.tile_pool(name="big", bufs=1))
        bandp = ctx.enter_context(tc.tile_pool(name="bandp", bufs=2))
        mulp = ctx.enter_context(tc.tile_pool(name="mulp", bufs=2))
        candp = ctx.enter_context(tc.tile_pool(name="candp", bufs=13))
        tmp = ctx.enter_context(tc.tile_pool(name="tmp", bufs=4))
        psum = ctx.enter_context(tc.tile_pool(name="psum", bufs=1, space="PSUM"))

        # ---------------- input loads ----------------
        atp_sb = big.tile([128, LT, NH, 48], bf16, tag="atp_sb")
        atp_r = atp_d.rearrange("(t p) (h j) -> p t h j", p=128, h=NH)
        ato_sb = big.tile([128, LT, NH, 48], bf16, tag="ato_sb")
        ato_r = ato_d.rearrange("(t p) (h j) -> p t h j", p=128, h=NH)
        for lt in range(LT):
            nc.sync.dma_start(out=atp_sb[:, lt], in_=atp_r[:, lt])
            nc.sync.dma_start(out=ato_sb[:, lt], in_=ato_r[:, lt])
        seq_sb = big.tile([128, LT, H], bf16, tag="seq_sb")
        nc.sync.dma_start(out=seq_sb, in_=seq_d.rearrange("(t p) n -> p t n", p=128))
        entT_sb = const.tile([128, HT, E], bf16, tag="entT_sb")
        nc.sync.dma_start(out=entT_sb, in_=entT_d.rearrange("(t p) n -> p t n", p=128))
        wh_sb = big.tile([128, 2 * HT, H], bf16, tag="wh_sb")
        nc.sync.dma_start(out=wh_sb, in_=wh_d.rearrange("(t p) n -> p t n", p=128))
        wt_sb = big.tile([128, 2 * HT, H], bf16, tag="wt_sb")
        nc.sync.dma_start(out=wt_sb, in_=wt_d.rearrange("(t p) n -> p t n", p=128))
        ptT_sb = const.tile([128, 2 * HT, RP], bf16, tag="ptT_sb")
        nc.sync.dma_start(out=ptT_sb, in_=ptT_d.rearrange("(t p) n -> p t n", p=128))

        ones_col = const.tile([128, 1], bf16, tag="ones_col")
        nc.vector.memset(ones_col, 1.0)
        ones_row = const.tile([1, 128], f32, tag="ones_row")
        nc.vector.memset(ones_row, 1.0)
        ident = const.tile([RP, RP], f32, tag="ident")
        make_identity(nc, ident)

        # ---------------- band products + fold + ctx/S accumulate ----------
        ctx_ps = [psum.tile([128, NB], f32, tag="ctx", bufs=HT,
                            name=f"ctx{ht}") for ht in range(HT)]
        # one aux bank: ctx-strip [6*32] at cols 0:192, S-strip at 192:224,
        # ep scratch at 224:256
        ctxs_ps = psum.tile([128, 256], f32, tag="ctxs", bufs=1, name="ctxs")
        s_ps = psum.tile([1, NB], f32, tag="s", bufs=1, name="s_ps")
        ep_sb = const.tile([128, 2, HT, 48], bf16, tag="ep_sb")

        def emit_ep():
            # entity projections ep[w][h'', e] = W[:H].T @ entT (interleaved
            # mid-loop: after wh/wt have landed, before PE runs dry)
            for w, wsb in ((0, wh_sb), (1, wt_sb)):
                for ht2 in range(HT):
                    ps = ctxs_ps[:, 224:256]
                    for kt in range(HT):
                        nc.tensor.matmul(
                            ps, wsb[:, kt, ht2 * 128:(ht2 + 1) * 128],
                            entT_sb[:, kt, :],
                            start=(kt == 0), stop=(kt == HT - 1))
                    nc.scalar.copy(ep_sb[:, w, ht2, :E], ps)
                    nc.scalar.copy(ep_sb[:, w, ht2, E:], ep_sb[:, w, ht2, :16])

        for lt in range(LT):
            xp = atp_sb[:, lt]                    # [128, NH, 48]
            xo = ato_sb[:, lt]
            b12 = bandp.tile([128, NH, NS], bf16, tag="b12", name=f"b12_{lt}")
            # in0: X[k] broadcast over 8 diagonals (outer stride 0, inner 1)
            x_b = xp[:, :, None, :E].broadcast_to([128, NH, 8, E])
            # even diagonals d = 0,2,..,14: in1 = X[k+d] (window stride 2)
            nc.vector.tensor_tensor(
                out=_win(b12, [[2 * E, 8], [1, E]]),
                in0=x_b, in1=_win(xp, [[2, 8], [1, E]]),
                op=Alu.mult)
            # odd diagonals d = 1,3,..,15: in1 = Xodd[k+d-1], Xodd[j]=X[j+1]
            nc.vector.tensor_tensor(
                out=_win(b12[:, :, E:], [[2 * E, 8], [1, E]]),
                in0=x_b, in1=_win(xo, [[2, 8], [1, E]]),
                op=Alu.mult)
            # strip d = 16: in1 = X[k+16]
            nc.vector.tensor_tensor(
                out=b12[:, :, NB:], in0=xp[:, :, :E], in1=xp[:, :, 16:],
                op=Alu.mult)
            # fold 12 heads -> band_mul
            nc.vector.tensor_add(b12[:, 0:6], b12[:, 0:6], b12[:, 6:12])
            nc.vector.tensor_add(b12[:, 0:3], b12[:, 0:3], b12[:, 3:6])
            nc.vector.tensor_add(b12[:, 0], b12[:, 0], b12[:, 1])
            bm = mulp.tile([128, NS], bf16, tag="bm", name=f"bm_{lt}")
            nc.vector.tensor_add(bm, b12[:, 0], b12[:, 2])

            # ctx += seq_lt.T @ band_mul ; S += ones.T @ band_mul
            st = (lt == 0)
            sp = (lt == LT - 1)
            for ht in range(HT):
                stat = seq_sb[:, lt, ht * 128:(ht + 1) * 128]
                nc.tensor.matmul(ctx_ps[ht], stat, bm[:, :NB],
                                 start=st, stop=sp)
                nc.tensor.matmul(ctxs_ps[:, ht * E:(ht + 1) * E], stat,
                                 bm[:, NB:], start=st, stop=sp)
            nc.tensor.matmul(s_ps, ones_col, bm[:, :NB], start=st, stop=sp)
            nc.tensor.matmul(ctxs_ps[0:1, 192:224], ones_col, bm[:, NB:],
                             start=st, stop=sp)
            if lt == 3:
                emit_ep()

        # ---------------- 1/S broadcast to all partitions ----------------
        s_sb = const.tile([1, NS], f32, tag="s_sb")
        nc.scalar.copy(s_sb[:, :NB], s_ps)
        nc.scalar.copy(s_sb[:, NB:], ctxs_ps[0:1, 192:224])
        nc.vector.reciprocal_approx_fast(out=s_sb, in_=s_sb)
        recS_sb = const.tile([128, NS], f32, tag="recS_sb")
        rb = psum.tile([128, NB], f32, tag="s", bufs=1, name="recB")
        nc.tensor.matmul(rb, ones_row, s_sb[:, :NB], start=True, stop=True)
        nc.scalar.copy(recS_sb[:, :NB], rb)
        rb2 = psum.tile([128, E], f32, tag="s", bufs=1, name="recB2")
        nc.tensor.matmul(rb2, ones_row, s_sb[:, NB:], start=True, stop=True)
        nc.scalar.copy(recS_sb[:, NB:], rb2)

        # ---------------- normalize: cn = ctx * (1/S), bf16 ----------------
        cn = big.tile([128, HT, NS], bf16, tag="cn")
        for ht in range(HT):
            nc.vector.tensor_mul(cn[:, ht, :NB], ctx_ps[ht],
                                 recS_sb[:, :NB])
        nc.vector.tensor_mul(
            cn[:, :, NB:],
            ctxs_ps[:, 0:192].rearrange("p (a b) -> p a b", a=HT),
            recS_sb[:, None, NB:].broadcast_to([128, HT, E]))

        # ---------------- proj + bias + tanh (both orientations) -----------
        candF = [None] * (2 * HT)
        candR = [None] * (2 * HT)
        for g in range(2 * HT):
            w, ht2 = divmod(g, HT)
            wsb = wh_sb if w == 0 else wt_sb
            ps = psum.tile([128, NB], f32, tag="ctx", bufs=HT,
                           name=f"proj{g}")
            pss = psum.tile([128, E], f32, tag="ctx", bufs=HT,
                            name=f"projs{g}")
            for kt in range(HT):
                stat = wsb[:, HT + kt, ht2 * 128:(ht2 + 1) * 128]
                nc.tensor.matmul(ps, stat, cn[:, kt, :NB],
                                 start=(kt == 0), stop=(kt == HT - 1))
                nc.tensor.matmul(pss, stat, cn[:, kt, NB:],
                                 start=(kt == 0), stop=(kt == HT - 1))
            pj = tmp.tile([128, NS], bf16, tag="pj", name=f"pj{g}")
            nc.scalar.copy(pj[:, :NB], ps)
            nc.scalar.copy(pj[:, NB:], pss)

            epw = ep_sb[:, w, ht2]
            # fwd pair (k, k+d): head bias = ep[k], tail bias = ep[k+d]
            # (strip row d=16 included -> 17 "diagonals")
            if w == 0:
                bias_f = _win(epw, [[0, ND + 1], [1, E]])
                bias_r = _win(epw, [[1, ND], [1, E]])
            else:
                bias_f = _win(epw, [[1, ND + 1], [1, E]])
                bias_r = _win(epw, [[0, ND], [1, E]])
            pre_f = tmp.tile([128, NS], bf16, tag="pre", name=f"pref{g}")
            nc.vector.tensor_add(
                pre_f.rearrange("p (a b) -> p a b", a=ND + 1),
                pj.rearrange("p (a b) -> p a b", a=ND + 1), bias_f)
            cf = candp.tile([128, NS], bf16, tag="candF", name=f"candF{g}")
            candF[g] = cf
            nc.scalar.activation(cf, pre_f, Act.Tanh)

            pre_r = tmp.tile([128, NB], bf16, tag="pre", name=f"prer{g}")
            nc.vector.tensor_add(
                pre_r.rearrange("p (a b) -> p a b", a=ND),
                pj[:, :NB].rearrange("p (a b) -> p a b", a=ND), bias_r)
            cr = candp.tile([128, NB], bf16, tag="candR", name=f"candR{g}")
            candR[g] = cr
            nc.scalar.activation(cr, pre_r, Act.Tanh)

        # ---------------- scores + per-class max ----------------
        scf_ps = psum.tile([RP, NB], f32, tag="s", bufs=1, name="scf")
        scs_ps = psum.tile([RP, E], f32, tag="ctxs", bufs=1, name="scs")
        scr_ps = psum.tile([RP, NB], f32, tag="ctx", bufs=HT, name="scr")
        for g in range(2 * HT):
            st = (g == 0)
            sp = (g == 2 * HT - 1)
            nc.tensor.matmul(scf_ps, ptT_sb[:, g, :], candF[g][:, :NB],
                             start=st, stop=sp)
            nc.tensor.matmul(scs_ps, ptT_sb[:, g, :], candF[g][:, NB:],
                             start=st, stop=sp)
            nc.tensor.matmul(scr_ps, ptT_sb[:, g, :], candR[g],
                             start=st, stop=sp)
        scT = const.tile([RP, 2 * NB + E], f32, tag="scT")
        nc.scalar.copy(scT[:, :NB], scf_ps)
        nc.scalar.copy(scT[:, NB:2 * NB], scr_ps)
        nc.scalar.copy(scT[:, 2 * NB:], scs_ps)

        obF = const.tile([128, 4, R], f32, tag="obF")
        obR = const.tile([128, 4, R], f32, tag="obR")
        obS = const.tile([E, R], f32, tag="obS")
        for c in range(9):
            wdt = E if c == 8 else 128
            tp = psum.tile([wdt, RP], f32, tag="ctx", bufs=HT, name=f"tp{c}")
            nc.tensor.transpose(tp, scT[:, c * 128:c * 128 + wdt], ident)
            if c == 8:
                dst = obS[:, None, :]
            elif c < 4:
                dst = obF[:, c, None, :]
            else:
                dst = obR[:, c - 4, None, :]
            nc.vector.tensor_reduce(
                out=dst.rearrange("p a b -> p (a b)"),
                in_=tp.rearrange("p (r q) -> p r q", r=R),
                axis=Ax.X, op=Alu.max)

        out_r = out_d.rearrange("(c p) r -> p c r", p=128)
        nc.scalar.dma_start(out=out_r[:, 0:4], in_=obF)
        nc.scalar.dma_start(out=out_r[:, 4:8], in_=obR)
        nc.scalar.dma_start(out=out_d[1024:1056].rearrange(
            "(a p) r -> p a r", p=E), in_=obS[:, None, :])


def _host_prep(sequence_output, attention, W_head, W_tail, prototypes,
               mention_pos):
    """Build the per-core input maps (numpy indexing / dtype conversion)."""
    import ml_dtypes
    bf = ml_dtypes.bfloat16

    seq = np.asarray(sequence_output, dtype=np.float32)
    att = np.asarray(attention, dtype=np.float32)
    wh = np.ascontiguousarray(W_head, dtype=np.float32).astype(bf)
    wt = np.ascontiguousarray(W_tail, dtype=np.float32).astype(bf)
    pro = np.asarray(prototypes, dtype=np.float32)
    pos = np.asarray(mention_pos)

    in_maps = []
    for c in range(NCORES):
        b, q = divmod(c, Q)
        p_bq = pos[b, q]                       # [E, M]
        # attention gather + mention-sum: At[l, h, e]  (scale dropped)
        g = att[b, q][:, p_bq, :]              # [NH, E, M, L]
        asum = (g[:, :, 0, :] + g[:, :, 1, :])  # [NH, E, L]
        at = np.ascontiguousarray(np.transpose(asum, (2, 0, 1)))  # [L,NH,E]
        atp = np.concatenate([at, at[:, :, :16]], axis=2)  # [L, NH, 48]
        ato = np.zeros_like(atp)
        ato[:, :, :47] = atp[:, :, 1:]
        # entity means: ent[e] = mean_m seq[pos]  -> entT [H, E]
        ment = seq[b, q][p_bq]                 # [E, M, H]
        ent = (ment[:, 0, :] + ment[:, 1, :]) * np.float32(0.5)
        entT = np.ascontiguousarray(ent.T)
        ptT = np.ascontiguousarray(pro[b].reshape(RP, 2 * H).T)
        in_maps.append({
            "atp": atp.reshape(L, NH * 48).astype(bf),
            "ato": ato.reshape(L, NH * 48).astype(bf),
            "seq": seq[b, q].astype(bf),
            "entT": entT.astype(bf),
            "wh": wh,
            "wt": wt,
            "ptT": ptT.astype(bf),
        })
    return in_maps


# band slot -> grid index maps (precomputed once)
def _unband_index():
    dd, kk = np.meshgrid(np.arange(ND), np.arange(E), indexing="ij")
    f_fwd = (kk + dd) % E
    # fwd slot (d, k) -> (k, (k+d)%32); rev slot -> ((k+d)%32, k)
    fwd_e, fwd_f = kk.ravel(), f_fwd.ravel()
    rev_e, rev_f = f_fwd.ravel(), kk.ravel()
    k = np.arange(E)
    strip_e, strip_f = k, (k + 16) % E
    return fwd_e, fwd_f, rev_e, rev_f, strip_e, strip_f


def kernel(sequence_output, attention, W_head, W_tail, prototypes,
           mention_pos):
    from concourse.bass_utils import run_bass_kernel_spmd

    if "nc" not in _CACHE:
        _CACHE["nc"] = _build_program()
        _CACHE["idx"] = _unband_index()
    nc = _CACHE["nc"]
    fwd_e, fwd_f, rev_e, rev_f, strip_e, strip_f = _CACHE["idx"]

    in_maps = _host_prep(sequence_output, attention, W_head, W_tail,
                         prototypes, mention_pos)
    res = run_bass_kernel_spmd(nc, in_maps, core_ids=list(range(NCORES)))

    out = np.empty((B, Q, E, E, R), dtype=np.float32)
    for c in range(NCORES):
        b, q = divmod(c, Q)
        raw = res.results[c]["out"]
        grid = out[b, q]
        grid[rev_e, rev_f] = raw[512:1024]
        grid[fwd_e, fwd_f] = raw[0:512]
        grid[strip_e, strip_f] = raw[1024:1056]
    return out


# revision 26
# speedup vs baseline: 1.0788x; 1.0147x over previous
"""Trainium2 Bass kernel for nn_BaseEncoder (ragged entity-pair encoder).

Contract: kernel(**inputs) takes the FULL unsharded inputs (numpy) and
returns the FULL output [B, Q, E, E, R] float32.

Sharding: B*Q = 8 independent (batch, query) pairs -> one per NeuronCore.

Diagonal-band formulation (v2): the pair tensor mul[l, e, f] is symmetric in
(e, f), so the device computes it only on 544 unique "band" slots:
  main band: slot (d, k), d in 0..15, k in 0..31  -> pair (k, (k+d) % 32)
  strip:     slot k,      k in 0..31              -> pair (k, (k+16) % 32)
Band products use overlapping-window / outer-broadcast access patterns whose
inner stride is 1, which keeps the DVE in its 2x bf16 perf mode (the previous
grid formulation's inner-stride-0 broadcasts ran at 1x).  ctx / S / norm /
proj all run once on the shared symmetric band; only bias+tanh+scores are
duplicated per pair orientation (fwd = (k, k+d), rev = (k+d, k)).  The final
band -> [E, E] grid reorder is pure indexing done on the host.

Per-head products go to band12[h]; an add tree folds the 12 heads.  Odd
diagonals read from a host-supplied one-element-shifted copy of the padded
attention rows so every window is 4-byte aligned (bf16 2x mode needs that).
"""

import numpy as np

B, Q, L, H, E, M, R, P, NH = 2, 4, 1024, 768, 32, 2, 5, 10, 12
NCORES = 8
LT = L // 128          # 8 l-tiles
HT = H // 128          # 6 tiles of 128 along a hidden dim
RP = R * P             # 50 prototype rows
ND = 16                # main band diagonals (d = 0..15)
NB = ND * E            # 512 main band slots (= one fp32 PSUM bank)
NS = NB + E            # 544 slots incl. the distance-16 strip
OUTROWS = 1152         # 512 fwd + 512 rev + 32 strip + 96 pad (9 * 128)

_CACHE = {}


def _build_program():
    import concourse.mybir as mybir
    import concourse.tile as tile
    from concourse import bacc

    f32 = mybir.dt.float32
    bf16 = mybir.dt.bfloat16
    nc = bacc.Bacc("TRN2", target_bir_lowering=False, debug=False,
                   num_devices=NCORES)

    atp_d = nc.dram_tensor("atp", [L, NH * 48], bf16, kind="ExternalInput").ap()
    ato_d = nc.dram_tensor("ato", [L, NH * 48], bf16, kind="ExternalInput").ap()
    seq_d = nc.dram_tensor("seq", [L, H], bf16, kind="ExternalInput").ap()
    entT_d = nc.dram_tensor("entT", [H, E], bf16, kind="ExternalInput").ap()
    wh_d = nc.dram_tensor("wh", [2 * H, H], bf16, kind="ExternalInput").ap()
    wt_d = nc.dram_tensor("wt", [2 * H, H], bf16, kind="ExternalInput").ap()
    ptT_d = nc.dram_tensor("ptT", [2 * H, RP], bf16, kind="ExternalInput").ap()
    out_d = nc.dram_tensor("out", [OUTROWS, R], f32, kind="ExternalOutput").ap()

    with tile.TileContext(nc) as tc:
        _emit(tc, mybir, atp_d, ato_d, seq_d, entT_d, wh_d, wt_d, ptT_d, out_d)

    nc.compile()
    return nc


def _win(ap, dims):
    """Replace the trailing free dim of `ap` with explicit [stride, size]
    dims (overlapping windows allowed)."""
    a = ap.copy()
    a.ap = a.ap[:-1] + [list(d) for d in dims]
    return a


def _emit(tc, mybir, atp_d, ato_d, seq_d, entT_d, wh_d, wt_d, ptT_d, out_d):
    nc = tc.nc
    f32 = mybir.dt.float32
    bf16 = mybir.dt.bfloat16
    Alu = mybir.AluOpType
    Act = mybir.ActivationFunctionType
    Ax = mybir.AxisListType
    from concourse.masks import make_identity

    import contextlib
    ctx = contextlib.ExitStack()
    with ctx:
        const = ctx.enter_context(tc.tile_pool(name="const", bufs=1))
        big = ctx.enter_context(tc.tile_pool(name="big", bufs=1))
        bandp = ctx.enter_context(tc.tile_pool(name="bandp", bufs=2))
        mulp = ctx.enter_context(tc.tile_pool(name="mulp", bufs=2))
        candp = ctx.enter_context(tc.tile_pool(name="candp", bufs=13))
        tmp = ctx.enter_context(tc.tile_pool(name="tmp", bufs=4))
        psum = ctx.enter_context(tc.tile_pool(name="psum", bufs=1, space="PSUM"))

        # ---------------- input loads ----------------
        atp_sb = big.tile([128, LT, NH, 48], bf16, tag="atp_sb")
        atp_r = atp_d.rearrange("(t p) (h j) -> p t h j", p=128, h=NH)
        ato_sb = big.tile([128, LT, NH, 48], bf16, tag="ato_sb")
        ato_r = ato_d.rearrange("(t p) (h j) -> p t h j", p=128, h=NH)
        for lt in range(LT):
            nc.sync.dma_start(out=atp_sb[:, lt], in_=atp_r[:, lt])
            nc.sync.dma_start(out=ato_sb[:, lt], in_=ato_r[:, lt])
        seq_sb = big.tile([128, LT, H], bf16, tag="seq_sb")
        nc.sync.dma_start(out=seq_sb, in_=seq_d.rearrange("(t p) n -> p t n", p=128))
        entT_sb = const.tile([128, HT, E], bf16, tag="entT_sb")
        nc.sync.dma_start(out=entT_sb, in_=entT_d.rearrange("(t p) n -> p t n", p=128))
        # weight DMAs are triggered from the scalar queue, gated on loop
        # progress (emitted at lt==2 below) to keep them out of the
        # product loop's startup window
        wh_sb = big.tile([128, 2 * HT, H], bf16, tag="wh_sb")
        wt_sb = big.tile([128, 2 * HT, H], bf16, tag="wt_sb")
        ptT_sb = const.tile([128, 2 * HT, RP], bf16, tag="ptT_sb")
        gate_t = const.tile([1, 1], bf16, tag="gate_t")

        ones_col = const.tile([128, 1], bf16, tag="ones_col")
        nc.vector.memset(ones_col, 1.0)
        ones_row = const.tile([1, 128], f32, tag="ones_row")
        nc.vector.memset(ones_row, 1.0)
        ident = const.tile([RP, RP], f32, tag="ident")
        make_identity(nc, ident)

        # ---------------- band products + fold + ctx/S accumulate ----------
        ctx_ps = [psum.tile([128, NB], f32, tag="ctx", bufs=HT,
                            name=f"ctx{ht}") for ht in range(HT)]
        # one aux bank: ctx-strip [6*32] at cols 0:192, S-strip at 192:224,
        # ep scratch at 224:256
        ctxs_ps = psum.tile([128, 256], f32, tag="ctxs", bufs=1, name="ctxs")
        s_ps = psum.tile([1, NB], f32, tag="s", bufs=1, name="s_ps")
        ep_sb = const.tile([128, 2, HT, 48], bf16, tag="ep_sb")

        def emit_ep():
            # entity projections ep[w][h'', e] = W[:H].T @ entT (interleaved
            # mid-loop: after wh/wt have landed, before PE runs dry)
            for w, wsb in ((0, wh_sb), (1, wt_sb)):
                for ht2 in range(HT):
                    ps = ctxs_ps[:, 224:256]
                    for kt in range(HT):
                        nc.tensor.matmul(
                            ps, wsb[:, kt, ht2 * 128:(ht2 + 1) * 128],
                            entT_sb[:, kt, :],
                            start=(kt == 0), stop=(kt == HT - 1))
                    nc.scalar.copy(ep_sb[:, w, ht2, :E], ps)
                    nc.scalar.copy(ep_sb[:, w, ht2, E:], ep_sb[:, w, ht2, :16])

        for lt in range(LT):
            xp = atp_sb[:, lt]                    # [128, NH, 48]
            xo = ato_sb[:, lt]
            b12 = bandp.tile([128, NH, NS], bf16, tag="b12", name=f"b12_{lt}")
            # in0: X[k] broadcast over 8 diagonals (outer stride 0, inner 1)
            x_b = xp[:, :, None, :E].broadcast_to([128, NH, 8, E])
            # even diagonals d = 0,2,..,14: in1 = X[k+d] (window stride 2)
            nc.vector.tensor_tensor(
                out=_win(b12, [[2 * E, 8], [1, E]]),
                in0=x_b, in1=_win(xp, [[2, 8], [1, E]]),
                op=Alu.mult)
            # odd diagonals d = 1,3,..,15: in1 = Xodd[k+d-1], Xodd[j]=X[j+1]
            nc.vector.tensor_tensor(
                out=_win(b12[:, :, E:], [[2 * E, 8], [1, E]]),
                in0=x_b, in1=_win(xo, [[2, 8], [1, E]]),
                op=Alu.mult)
            # strip d = 16: in1 = X[k+16]
            nc.vector.tensor_tensor(
                out=b12[:, :, NB:], in0=xp[:, :, :E], in1=xp[:, :, 16:],
                op=Alu.mult)
            # fold 12 heads -> band_mul
            nc.vector.tensor_add(b12[:, 0:6], b12[:, 0:6], b12[:, 6:12])
            nc.vector.tensor_add(b12[:, 0:3], b12[:, 0:3], b12[:, 3:6])
            nc.vector.tensor_add(b12[:, 0], b12[:, 0], b12[:, 1])
            bm = mulp.tile([128, NS], bf16, tag="bm", name=f"bm_{lt}")
            nc.vector.tensor_add(bm, b12[:, 0], b12[:, 2])

            # ctx += seq_lt.T @ band_mul ; S += ones.T @ band_mul
            st = (lt == 0)
            sp = (lt == LT - 1)
            for ht in range(HT):
                stat = seq_sb[:, lt, ht * 128:(ht + 1) * 128]
                nc.tensor.matmul(ctx_ps[ht], stat, bm[:, :NB],
                                 start=st, stop=sp)
                nc.tensor.matmul(ctxs_ps[:, ht * E:(ht + 1) * E], stat,
                                 bm[:, NB:], start=st, stop=sp)
            nc.tensor.matmul(s_ps, ones_col, bm[:, :NB], start=st, stop=sp)
            nc.tensor.matmul(ctxs_ps[0:1, 192:224], ones_col, bm[:, NB:],
                             start=st, stop=sp)
            if lt == 2:
                nc.scalar.copy(gate_t, bm[0:1, 0:1])
                nc.scalar.dma_start(
                    out=wh_sb, in_=wh_d.rearrange("(t p) n -> p t n", p=128))
                nc.scalar.dma_start(
                    out=wt_sb, in_=wt_d.rearrange("(t p) n -> p t n", p=128))
                nc.scalar.dma_start(
                    out=ptT_sb, in_=ptT_d.rearrange("(t p) n -> p t n", p=128))

        emit_ep()

        # ---------------- 1/S broadcast to all partitions ----------------
        s_sb = const.tile([1, NS], f32, tag="s_sb")
        nc.scalar.copy(s_sb[:, :NB], s_ps)
        nc.scalar.copy(s_sb[:, NB:], ctxs_ps[0:1, 192:224])
        nc.vector.reciprocal_approx_fast(out=s_sb, in_=s_sb)
        recS_sb = const.tile([128, NS], f32, tag="recS_sb")
        rb = psum.tile([128, NB], f32, tag="s", bufs=1, name="recB")
        nc.tensor.matmul(rb, ones_row, s_sb[:, :NB], start=True, stop=True)
        nc.scalar.copy(recS_sb[:, :NB], rb)
        rb2 = psum.tile([128, E], f32, tag="s", bufs=1, name="recB2")
        nc.tensor.matmul(rb2, ones_row, s_sb[:, NB:], start=True, stop=True)
        nc.scalar.copy(recS_sb[:, NB:], rb2)

        # ---------------- normalize: cn = ctx * (1/S), bf16 ----------------
        cn = big.tile([128, HT, NS], bf16, tag="cn")
        for ht in range(HT):
            nc.vector.tensor_mul(cn[:, ht, :NB], ctx_ps[ht],
                                 recS_sb[:, :NB])
        nc.vector.tensor_mul(
            cn[:, :, NB:],
            ctxs_ps[:, 0:192].rearrange("p (a b) -> p a b", a=HT),
            recS_sb[:, None, NB:].broadcast_to([128, HT, E]))

        # ---------------- proj + bias + tanh (both orientations) -----------
        candF = [None] * (2 * HT)
        candR = [None] * (2 * HT)
        for g in range(2 * HT):
            w, ht2 = divmod(g, HT)
            wsb = wh_sb if w == 0 else wt_sb
            ps = psum.tile([128, NB], f32, tag="ctx", bufs=HT,
                           name=f"proj{g}")
            pss = psum.tile([128, E], f32, tag="ctx", bufs=HT,
                            name=f"projs{g}")
            for kt in range(HT):
                stat = wsb[:, HT + kt, ht2 * 128:(ht2 + 1) * 128]
                nc.tensor.matmul(ps, stat, cn[:, kt, :NB],
                                 start=(kt == 0), stop=(kt == HT - 1))
                nc.tensor.matmul(pss, stat, cn[:, kt, NB:],
                                 start=(kt == 0), stop=(kt == HT - 1))
            pj = tmp.tile([128, NS], bf16, tag="pj", name=f"pj{g}")
            nc.scalar.copy(pj[:, :NB], ps)
            nc.scalar.copy(pj[:, NB:], pss)

            epw = ep_sb[:, w, ht2]
            # fwd pair (k, k+d): head bias = ep[k], tail bias = ep[k+d]
            # (strip row d=16 included -> 17 "diagonals")
            if w == 0:
                bias_f = _win(epw, [[0, ND + 1], [1, E]])
                bias_r = _win(epw, [[1, ND], [1, E]])
            else:
                bias_f = _win(epw, [[1, ND + 1], [1, E]])
                bias_r = _win(epw, [[0, ND], [1, E]])
            pre_f = tmp.tile([128, NS], bf16, tag="pre", name=f"pref{g}")
            nc.vector.tensor_add(
                pre_f.rearrange("p (a b) -> p a b", a=ND + 1),
                pj.rearrange("p (a b) -> p a b", a=ND + 1), bias_f)
            cf = candp.tile([128, NS], bf16, tag="candF", name=f"candF{g}")
            candF[g] = cf
            nc.scalar.activation(cf, pre_f, Act.Tanh)

            pre_r = tmp.tile([128, NB], bf16, tag="pre", name=f"prer{g}")
            nc.vector.tensor_add(
                pre_r.rearrange("p (a b) -> p a b", a=ND),
                pj[:, :NB].rearrange("p (a b) -> p a b", a=ND), bias_r)
            cr = candp.tile([128, NB], bf16, tag="candR", name=f"candR{g}")
            candR[g] = cr
            nc.scalar.activation(cr, pre_r, Act.Tanh)

        # ---------------- scores + per-class max ----------------
        scf_ps = psum.tile([RP, NB], f32, tag="s", bufs=1, name="scf")
        scs_ps = psum.tile([RP, E], f32, tag="ctxs", bufs=1, name="scs")
        scr_ps = psum.tile([RP, NB], f32, tag="ctx", bufs=HT, name="scr")
        for g in range(2 * HT):
            st = (g == 0)
            sp = (g == 2 * HT - 1)
            nc.tensor.matmul(scf_ps, ptT_sb[:, g, :], candF[g][:, :NB],
                             start=st, stop=sp)
            nc.tensor.matmul(scs_ps, ptT_sb[:, g, :], candF[g][:, NB:],
                             start=st, stop=sp)
            nc.tensor.matmul(scr_ps, ptT_sb[:, g, :], candR[g],
                             start=st, stop=sp)
        scT = const.tile([RP, 2 * NB + E], f32, tag="scT")
        nc.scalar.copy(scT[:, :NB], scf_ps)
        nc.scalar.copy(scT[:, NB:2 * NB], scr_ps)
        nc.scalar.copy(scT[:, 2 * NB:], scs_ps)

        obF = const.tile([128, 4, R], f32, tag="obF")
        obR = const.tile([128, 4, R], f32, tag="obR")
        obS = const.tile([E, R], f32, tag="obS")
        for c in range(9):
            wdt = E if c == 8 else 128
            tp = psum.tile([wdt, RP], f32, tag="ctx", bufs=HT, name=f"tp{c}")
            nc.tensor.transpose(tp, scT[:, c * 128:c * 128 + wdt], ident)
            if c == 8:
                dst = obS[:, None, :]
            elif c < 4:
                dst = obF[:, c, None, :]
            else:
                dst = obR[:, c - 4, None, :]
            nc.vector.tensor_reduce(
                out=dst.rearrange("p a b -> p (a b)"),
                in_=tp.rearrange("p (r q) -> p r q", r=R),
                axis=Ax.X, op=Alu.max)

        out_r = out_d.rearrange("(c p) r -> p c r", p=128)
        nc.scalar.dma_start(out=out_r[:, 0:4], in_=obF)
        nc.scalar.dma_start(out=out_r[:, 4:8], in_=obR)
        nc.scalar.dma_start(out=out_d[1024:1056].rearrange(
            "(a p) r -> p a r", p=E), in_=obS[:, None, :])


def _host_prep(sequence_output, attention, W_head, W_tail, prototypes,
               mention_pos):
    """Build the per-core input maps (numpy indexing / dtype conversion)."""
    import ml_dtypes
    bf = ml_dtypes.bfloat16

    seq = np.asarray(sequence_output, dtype=np.float32)
    att = np.asarray(attention, dtype=np.float32)
    wh = np.ascontiguousarray(W_head, dtype=np.float32).astype(bf)
    wt = np.ascontiguousarray(W_tail, dtype=np.float32).astype(bf)
    pro = np.asarray(prototypes, dtype=np.float32)
    pos = np.asarray(mention_pos)

    in_maps = []
    for c in range(NCORES):
        b, q = divmod(c, Q)
        p_bq = pos[b, q]                       # [E, M]
        # attention gather + mention-sum: At[l, h, e]  (scale dropped)
        g = att[b, q][:, p_bq, :]              # [NH, E, M, L]
        asum = (g[:, :, 0, :] + g[:, :, 1, :])  # [NH, E, L]
        at = np.ascontiguousarray(np.transpose(asum, (2, 0, 1)))  # [L,NH,E]
        atp = np.concatenate([at, at[:, :, :16]], axis=2)  # [L, NH, 48]
        ato = np.zeros_like(atp)
        ato[:, :, :47] = atp[:, :, 1:]
        # entity means: ent[e] = mean_m seq[pos]  -> entT [H, E]
        ment = seq[b, q][p_bq]                 # [E, M, H]
        ent = (ment[:, 0, :] + ment[:, 1, :]) * np.float32(0.5)
        entT = np.ascontiguousarray(ent.T)
        ptT = np.ascontiguousarray(pro[b].reshape(RP, 2 * H).T)
        in_maps.append({
            "atp": atp.reshape(L, NH * 48).astype(bf),
            "ato": ato.reshape(L, NH * 48).astype(bf),
            "seq": seq[b, q].astype(bf),
            "entT": entT.astype(bf),
            "wh": wh,
            "wt": wt,
            "ptT": ptT.astype(bf),
        })
    return in_maps


# band slot -> grid index maps (precomputed once)
def _unband_index():
    dd, kk = np.meshgrid(np.arange(ND), np.arange(E), indexing="ij")
    f_fwd = (kk + dd) % E
    # fwd slot (d, k) -> (k, (k+d)%32); rev slot -> ((k+d)%32, k)
    fwd_e, fwd_f = kk.ravel(), f_fwd.ravel()
    rev_e, rev_f = f_fwd.ravel(), kk.ravel()
    k = np.arange(E)
    strip_e, strip_f = k, (k + 16) % E
    return fwd_e, fwd_f, rev_e, rev_f, strip_e, strip_f


def kernel(sequence_output, attention, W_head, W_tail, prototypes,
           mention_pos):
    from concourse.bass_utils import run_bass_kernel_spmd

    if "nc" not in _CACHE:
        _CACHE["nc"] = _build_program()
        _CACHE["idx"] = _unband_index()
    nc = _CACHE["nc"]
    fwd_e, fwd_f, rev_e, rev_f, strip_e, strip_f = _CACHE["idx"]

    in_maps = _host_prep(sequence_output, attention, W_head, W_tail,
                         prototypes, mention_pos)
    res = run_bass_kernel_spmd(nc, in_maps, core_ids=list(range(NCORES)))

    out = np.empty((B, Q, E, E, R), dtype=np.float32)
    for c in range(NCORES):
        b, q = divmod(c, Q)
        raw = res.results[c]["out"]
        grid = out[b, q]
        grid[rev_e, rev_f] = raw[512:1024]
        grid[fwd_e, fwd_f] = raw[0:512]
        grid[strip_e, strip_f] = raw[1024:1056]
    return out
